# revision 27
# baseline (speedup 1.0000x reference)
"""Trainium2 Bass kernel for nn_Attention_interaction (dense_transformer).

Math (per batch b, head h):
    q = l2norm(x);  S = (q @ q^T) / SCALE / attn_gamma;  P = softmax(S, -1)
    o = P @ y;  o2 = o @ W^T + bias;  out = w0*y + w1*o2
with w_i = exp(sum_gamma_i) / (exp(sum_gamma0) + exp(sum_gamma1)).

Sharding: batch dim B=8 across the 8 cores (1 batch x 8 heads per core).
Per core the 8 heads run in 4 pairs (head A's qT operands on SBUF
partitions 0-63, head B's on 64-127, addressed via matmul tile_position).

The kernel is ACT(exp)-bound: softmax needs 8.4M exps per core; the Scalar
engine runs 1 elem/lane/cycle at 1.2 GHz, so the 64 [128,1024]-chunk
ACTIVATEs floor at ~74us. Everything else is arranged to keep that stream
dense:
  - Softmax skips max-subtraction (q rows are unit vectors so logits are
    bounded) and exp needs no accumulator: softmax denominators accumulate
    in the O matmul's 65th output row via a ones-column appended to y, and
    ride the proj matmul into per-partition layout via a unit column
    appended to the proj weight (out col 64 of each 128-col block = r).
  - Per head the S columns are jc-major and streamed through [128,1024]
    PSUM chunks (2-slot round robin); the two heads' S matmuls interleave
    so their disjoint PE row-quadrants overlap; O matmuls trail one chunk
    so the static per-engine order is always runnable.
  - x for all 8 heads is loaded once; the l2norm scales are computed with
    a handful of wide DVE ops per pair (fast-inverse-sqrt + 2 Newton).
  - All input/output DRAM is partition-major (the host pre/post-permutes)
    so every DMA moves long contiguous runs; y loads and stores ride the
    GpSimd software-DGE queue, the sync HWDGE queue carries the q
    transposes (pair 0's split onto the scalar HWDGE queue too).
  - PE and the exp table are warmed at t=0 with dummy matmuls / a dummy
    ACTIVATE. Pair-0 prep runs at scheduler priority 0, later pairs'
    norms are serialized behind it via bufs=1 scratch tiles, and rs is
    one tile per pair - all to keep the greedy per-engine list scheduler
    from hoisting non-critical work into the fill's dependency chain.
  - proj/denominator/epilogue are split per jc-half and spread across
    chunks (proj0 c=9/10, epilogue0 c=11/12); each pair's jc1 half is
    deferred into the NEXT pair's chunk stream (steps at its c=0..4), so
    pair boundaries cost <1us and only the final pair has a real tail
    (which scales o2 on the by-then-idle Scalar engine, all
    PE/DVE-local, no DRAM hops).
"""

import math
import os

import numpy as np
import ml_dtypes

import concourse.bass as bass
import concourse.bacc as bacc
import concourse.tile as tile
from concourse import mybir
from concourse.bass_utils import run_bass_kernel_spmd
from concourse._compat import get_trn_type

B, H, N, D = 8, 8, 1024, 64
SCALE = (512 // 8) ** (-0.5)  # 0.125
EPS = 1e-6
NCORES = 8
NB = N // 128  # 8 row blocks of 128
NW = N * NB  # 8192 flattened S columns per head
CHUNK = 1024  # exp granularity (PSUM columns per ACT instruction)
F32 = mybir.dt.float32
BF16 = mybir.dt.bfloat16
I32 = mybir.dt.int32
AX = mybir.AxisListType
OP = mybir.AluOpType
ACT = mybir.ActivationFunctionType
MAGIC = 0x5F3759DF

LAST_RESULTS = None  # BassKernelResults of the most recent run (for test.py)


def _emit(ctx, tc, sqrt_c2: float):
    """Emit the per-core program. sqrt_c2 = sqrt(1/(SCALE*attn_gamma)) is
    folded into the q row scales so S comes out of the PE pre-scaled."""
    nc = tc.nc
    # all DRAM I/O is partition-major ([h, p, ...]) so every DMA moves
    # long contiguous runs; the host does the (b p) <-> p b permutes.
    x_bf = nc.dram_tensor("x_bf", [H, 128, NB, D], BF16, kind="ExternalInput")
    ya = nc.dram_tensor("ya", [H, 128, NB, D + 1], BF16, kind="ExternalInput")
    yb = nc.dram_tensor("yb", [H, 128, NB, D], F32, kind="ExternalInput")
    wt = nc.dram_tensor("wt", [D + 1, D + 1], BF16, kind="ExternalInput")
    out = nc.dram_tensor("out", [H, 128, NB, D], F32, kind="ExternalOutput")

    singles = ctx.enter_context(tc.tile_pool(name="singles", bufs=1))
    # bufs=3: ypre(p+1) (emitted at pair p c==0) must not overwrite the
    # buffer pair p-1's tail O-flush is still reading
    io = ctx.enter_context(tc.tile_pool(name="io", bufs=3))
    # bufs=1: successive pairs' norms chains share buffers, so the WAR
    # deps stop the greedy scheduler from hoisting a later pair's norms
    # into the current critical chain
    st = ctx.enter_context(tc.tile_pool(name="st", bufs=1))
    work = ctx.enter_context(tc.tile_pool(name="work", bufs=2))
    epool = ctx.enter_context(tc.tile_pool(name="epool", bufs=2))
    qpool = ctx.enter_context(tc.tile_pool(name="qpool", bufs=1))
    # PSUM: 8 banks = S stream 2x[128,1024] (4) + per-head jc0/jc1 banks (4)
    ps_s = ctx.enter_context(tc.tile_pool(name="ps_s", bufs=2, space="PSUM"))
    ps_o = ctx.enter_context(tc.tile_pool(name="ps_o", bufs=1, space="PSUM"))

    # x for all heads, resident for the whole kernel (8KB/partition);
    # pair 0's heads load first and alone so their transfer isn't queued
    # behind anything
    xall = singles.tile([128, H, NB, D], BF16)
    nc.sync.dma_start(out=xall[:, 0, 0:4], in_=x_bf[0][:, 0:4])
    nc.sync.dma_start(out=xall[:, 1, 0:4], in_=x_bf[1][:, 0:4])
    nc.sync.dma_start(out=xall[:, 0, 4:NB], in_=x_bf[0][:, 4:NB])
    nc.sync.dma_start(out=xall[:, 1, 4:NB], in_=x_bf[1][:, 4:NB])

    # proj weight (rows 0-63 = w1*W^T, row 64 = w1*bias; col 64 = e_r so
    # each proj block's output col 64 is the softmax denominator)
    wt_sb = singles.tile([D + 1, D + 1], BF16)
    nc.sync.dma_start(out=wt_sb, in_=wt[:, :])

    ytiles = {}

    def ypre(p):
        """Prefetch pair p's y operands on the gpsimd software-DGE queue."""
        hA, hB = 2 * p, 2 * p + 1
        yA = io.tile([128, NB, D + 1], BF16, tag="yA", name=f"yA{p}")
        yB = io.tile([128, NB, D + 1], BF16, tag="yB", name=f"yB{p}")
        ybA = io.tile([128, NB, D], F32, tag="ybA", name=f"ybA{p}")
        ybB = io.tile([128, NB, D], F32, tag="ybB", name=f"ybB{p}")
        nc.gpsimd.dma_start(out=yA, in_=ya[hA])
        nc.gpsimd.dma_start(out=yB, in_=ya[hB])
        nc.gpsimd.dma_start(out=ybA, in_=yb[hA])
        nc.gpsimd.dma_start(out=ybB, in_=yb[hB])
        ytiles[p] = (yA, yB, ybA, ybB)

    # row scales rs[p, h, b] = sqrt_c2 / sqrt(sum_d x^2 + eps); one tile
    # per pair so a pair's qscale never waits on later pairs' norms writes
    rs = [
        singles.tile([128, 2, NB], F32, name=f"rs{p}") for p in range(H // 2)
    ]

    # ---- t=0 warmup: exp-table load + PE HAM un-throttle --------------
    scr = singles.tile([128, 2], F32)
    nc.vector.memset(scr, 0.0)
    nc.scalar.activation(out=scr[:, 1:2], in_=scr[:, 0:1], func=ACT.Exp)
    ps_warm = ps_s.tile([128, CHUNK], F32, tag="psS", name="warm")
    for _ in range(60):
        nc.tensor.matmul(
            ps_warm[0:D, 0:D],
            lhsT=wt_sb[0:D, 0:D],
            rhs=wt_sb[0:D, 0:D],
            start=True, stop=True, tile_position=(0, 0),
        )

    def norms(p, b0, nb):
        """l2norm scale for pair p's two heads, blocks b0..b0+nb: a short
        chain of wide DVE ops (square, reduce, fast-inverse-sqrt + 2
        Newton). The last Newton multiply lands directly in rs."""
        hA = 2 * p
        xs = xall[:, hA : hA + 2, b0 : b0 + nb, :]
        rsp = rs[p][:, :, b0 : b0 + nb]
        sq = st.tile([128, 2, NB, D], F32, tag="sq", name="sq")
        sq = sq[:, :, b0 : b0 + nb, :]
        ss = st.tile([128, 2, NB], F32, tag="ss", name="ss")[:, :, b0 : b0 + nb]
        half = st.tile([128, 2, NB], F32, tag="half", name="half")
        half = half[:, :, b0 : b0 + nb]
        yv = st.tile([128, 2, NB], F32, tag="yv", name="yv")[:, :, b0 : b0 + nb]
        t1 = st.tile([128, 2, NB], F32, tag="t1", name="t1")[:, :, b0 : b0 + nb]
        nc.vector.tensor_mul(sq, xs, xs)
        nc.vector.reduce_sum(ss, sq, axis=AX.X)
        nc.vector.tensor_scalar(
            out=half, in0=ss, scalar1=0.5, scalar2=0.5 * EPS,
            op0=OP.mult, op1=OP.add,
        )
        yi = yv.bitcast(I32)
        nc.vector.tensor_scalar(
            out=yi, in0=ss.bitcast(I32), scalar1=1, scalar2=None,
            op0=OP.logical_shift_right,
        )
        nc.vector.tensor_scalar(
            out=yi, in0=yi, scalar1=MAGIC, scalar2=-1,
            op0=OP.subtract, op1=OP.mult,
        )
        for it in range(1):
            last = it == 0
            nc.vector.tensor_mul(t1, yv, yv)
            nc.vector.tensor_mul(t1, t1, half)
            nc.vector.tensor_scalar(
                out=t1, in0=t1, scalar1=1.5,
                scalar2=(-sqrt_c2 if last else -1.0),
                op0=OP.subtract, op1=OP.mult,
            )
            nc.vector.tensor_mul(rsp if last else yv, yv, t1)

    qT = [None] * (H // 2)
    qABs = {}

    def qscale_block(p, b):
        """Scale+cast x block b of pair p into the interleaved [A|B] qAB
        layout (two tensor_scalar ops)."""
        hA, hB = 2 * p, 2 * p + 1
        if p not in qABs:
            qABs[p] = work.tile(
                [128, NB, 128], BF16, tag=f"qAB{p % 2}", name=f"qAB{p}"
            )
        qAB = qABs[p]
        nc.vector.tensor_scalar_mul(
            out=qAB[:, b, 0:D], in0=xall[:, hA, b],
            scalar1=rs[p][:, 0, b : b + 1],
        )
        nc.vector.tensor_scalar_mul(
            out=qAB[:, b, D:128], in0=xall[:, hB, b],
            scalar1=rs[p][:, 1, b : b + 1],
        )

    def qtranspose_block(p, b, engine=None):
        if qT[p] is None:
            qT[p] = qpool.tile([128, N], BF16, tag=f"qT{p}", name=f"qT{p}")
        (engine or nc.sync).dma_start(
            out=qT[p][:, b * 128 : (b + 1) * 128], in_=qABs[p][:, b],
            transpose=True,
        )

    def prep_pair0():
        """Minimal-latency prep of pair 0 at scheduler priority 0, block
        group 0 first (enough for chunk c=0), its transposes split across
        the sync/scalar HWDGE queues."""
        with tc.high_priority():
            norms(0, 0, NB)
            for b in range(4):
                qscale_block(0, b)
                eng = nc.scalar if b % 2 == 1 else nc.sync
                qtranspose_block(0, b, eng)

    prev_tail = None
    prep_pair0()
    ypre(0)
    for h in range(2, H):
        nc.gpsimd.dma_start(out=xall[:, h], in_=x_bf[h])

    for p in range(H // 2):
        hA, hB = 2 * p, 2 * p + 1
        yA, yB, ybA, ybB = ytiles[p]
        q = qT[p]
        # E[p, chunk, head, 512]: chunk c=jc*8+i holds both heads' exp(S)
        # for (i-block rows, jc-half cols)
        E = epool.tile([128, 16, 2, 512], BF16, tag="E", name=f"E{p}")
        OTA = work.tile([D + 1, N], BF16, tag="OTA")
        OTB = work.tile([D + 1, N], BF16, tag="OTB")
        heads = (
            (0, yA, OTA),
            (64, yB, OTB),
        )
        okptr = [0, 0]  # per head: next O matmul (jc-major index jc*8+i)
        otile = [None, None]

        # rtsb[p, jc, head, block-of-half] = softmax denominators
        rtsb = st.tile([128, 2, 2, NB // 2], F32, tag="rtsb")
        rinv = st.tile([128, 2, 2, NB // 2], F32, tag="rinv")

        def emit_o(
            hidx, climit, heads=heads, okptr=okptr, otile=otile, E=E,
            scopy=False,
        ):
            """Emit O matmuls whose E chunk (k < climit) is ready. The
            65th output row accumulates the softmax denominators."""
            base, ytile, OT = heads[hidx]
            hc = "AB"[hidx]
            while okptr[hidx] < min(16, climit):
                k = okptr[hidx]
                jc, i = k // NB, k % NB
                if i == 0:
                    otile[hidx] = ps_o.tile(
                        [128, 512], F32, tag=f"o{jc}{hc}", name=f"ot{jc}{hc}"
                    )
                nc.tensor.matmul(
                    otile[hidx][0 : D + 1, :],
                    lhsT=ytile[:, i, :],
                    rhs=E[:, k, hidx],
                    start=(i == 0), stop=(i == NB - 1), tile_position=(0, 0),
                )
                if i == NB - 1:
                    with tc.high_priority():
                        if scopy:
                            nc.scalar.copy(
                                OT[:, jc * 512 : (jc + 1) * 512],
                                otile[hidx][0 : D + 1, :],
                            )
                        else:
                            nc.vector.tensor_copy(
                                OT[:, jc * 512 : (jc + 1) * 512],
                                otile[hidx][0 : D + 1, :],
                            )
                okptr[hidx] += 1

        def emit_proj(hidx, jc, heads=heads, rtsb=rtsb):
            """proj for output blocks jc*4..jc*4+3 (needs OT cols of that jc
            half); lands in the jc bank this head just freed. Each block's
            output col 64 is the softmax denominator for its 128 tokens."""
            base, ytile, OT = heads[hidx]
            hc = "AB"[hidx]
            pj = ps_o.tile([128, 512], F32, tag=f"o{jc}{hc}", name=f"pj{jc}{hc}")
            for b in range(jc * 4, jc * 4 + 4):
                nc.tensor.matmul(
                    pj[:, (b - jc * 4) * 128 : (b - jc * 4) * 128 + D + 1],
                    lhsT=OT[:, b * 128 : (b + 1) * 128],
                    rhs=wt_sb,
                    start=True, stop=True, tile_position=(0, 0),
                )
            # denominators out of the bank right away (they're per-partition
            # already)
            with tc.high_priority():
                nc.vector.tensor_copy(
                    rtsb[:, jc, hidx],
                    pj.rearrange("p (b c) -> p b c", b=4)[:, :, D],
                )
            return pj

        def emit_rinv(jc, hidx, rtsb=rtsb, rinv=rinv):
            nc.vector.reciprocal(rinv[:, jc, hidx], rtsb[:, jc, hidx])

        pjs = [[None, None], [None, None]]  # [hidx][jc]

        def emit_epilogue(
            hidx, jc, pj, dma=None, hs=None, ybs=None, rinv=rinv, out=out,
            use_act=False,
        ):
            """o2 = pj/r for one jc half, + w0*y, and DMA out. use_act
            scales on the (by then idle) Scalar engine instead of DVE."""
            ho = (hs or (hA, hB))[hidx]
            ybt = (ybs or (ybA, ybB))[hidx]
            o2 = work.tile(
                [128, NB // 2, D], F32, tag=f"o2{'AB'[hidx]}{jc}",
                name=f"o2{'AB'[hidx]}{jc}",
            )
            for b in range(jc * 4, jc * 4 + 4):
                if use_act:
                    nc.scalar.activation(
                        out=o2[:, b - jc * 4, :],
                        in_=pj[:, (b % 4) * 128 : (b % 4) * 128 + D],
                        func=ACT.Copy,
                        scale=rinv[:, jc, hidx, b % 4 : b % 4 + 1],
                    )
                else:
                    nc.vector.tensor_scalar_mul(
                        out=o2[:, b - jc * 4, :],
                        in0=pj[:, (b % 4) * 128 : (b % 4) * 128 + D],
                        scalar1=rinv[:, jc, hidx, b % 4 : b % 4 + 1],
                    )
            fin = work.tile(
                [128, NB // 2, D], F32, tag=f"fin{'AB'[hidx]}{jc}",
                name=f"fin{'AB'[hidx]}{jc}",
            )
            nc.vector.tensor_add(fin, o2, ybt[:, jc * 4 : jc * 4 + 4, :])
            (dma or nc.gpsimd).dma_start(
                out=out[ho][:, jc * 4 : jc * 4 + 4], in_=fin
            )

        # ---- S/exp chunk stream with O interleaved (one-chunk delay) ----
        # Chunk c = jc*8+i holds both heads ([A 512 | B 512]); the two S
        # matmuls hit disjoint PE row-quadrants back-to-back and overlap.
        # Interleaved per chunk: next pair's norms/qscale/transposes and
        # the jc0 denominator/proj/epilogue.
        for c in range(16):
            jc, i = c // NB, c % NB
            ps = ps_s.tile([128, CHUNK], F32, tag="psS", name="psS")
            for hidx in range(2):
                base = heads[hidx][0]
                nc.tensor.matmul(
                    ps[:, hidx * 512 : (hidx + 1) * 512],
                    lhsT=q[base : base + 64, i * 128 : (i + 1) * 128],
                    rhs=q[base : base + 64, jc * 512 : (jc + 1) * 512],
                    start=True, stop=True, tile_position=(base, 0),
                )
            nc.scalar.activation(out=E[:, c], in_=ps, func=ACT.Exp)
            for hidx in range(2):
                emit_o(hidx, c)

            if p == 0:
                # rest of pair 0's own prep (blocks 4-7)
                if 1 <= c <= 4:
                    qscale_block(0, c + 3)
                    qtranspose_block(0, c + 3)
            if p + 1 < H // 2:
                if c == 0:
                    ypre(p + 1)
                elif c == 5:
                    norms(p + 1, 0, NB)
                elif 6 <= c <= 13:
                    b = c - 6
                    qscale_block(p + 1, b)
                    qtranspose_block(p + 1, b)
            if prev_tail is not None and c <= 4:
                prev_tail(c)
            if c == 9:
                pjs[0][0] = emit_proj(0, 0)
                emit_rinv(0, 0)
            elif c == 10:
                pjs[1][0] = emit_proj(1, 0)
                emit_rinv(0, 1)
            elif c == 11:
                emit_epilogue(0, 0, pjs[0][0])
            elif c == 12:
                emit_epilogue(1, 0, pjs[1][0])

        # ---- jc1 tail: for non-final pairs, deferred into the next
        # pair's chunk stream (steps at its c=0..4) ----
        def make_tail(
            emit_o, emit_proj, emit_rinv, emit_epilogue, pjs,
            hs=(hA, hB), ybs=None,
        ):
            def tail(c):
                if c == 0:
                    emit_o(0, 16)
                    emit_o(1, 16)
                elif c == 1:
                    pjs[0][1] = emit_proj(0, 1)
                    emit_rinv(1, 0)
                elif c == 2:
                    pjs[1][1] = emit_proj(1, 1)
                    emit_rinv(1, 1)
                elif c == 3:
                    emit_epilogue(0, 1, pjs[0][1], hs=hs, ybs=ybs)
                elif c == 4:
                    emit_epilogue(1, 1, pjs[1][1], hs=hs, ybs=ybs)
            return tail

        if p < H // 2 - 1:
            prev_tail = make_tail(
                emit_o, emit_proj, emit_rinv, emit_epilogue, pjs,
                hs=(hA, hB), ybs=(ybA, ybB),
            )
        else:
            emit_o(0, 16)
            emit_o(1, 16, scopy=True)
            pjs[0][1] = emit_proj(0, 1)
            emit_rinv(1, 0)
            pjs[1][1] = emit_proj(1, 1)
            emit_rinv(1, 1)
            emit_epilogue(0, 1, pjs[0][1], dma=nc.sync, use_act=True)
            emit_epilogue(1, 1, pjs[1][1], dma=nc.scalar)


def build_program(sqrt_c2: float) -> bass.Bass:
    from contextlib import ExitStack

    nc = bacc.Bacc(get_trn_type() or "TRN2", target_bir_lowering=False)
    with tile.TileContext(nc) as tc:
        with ExitStack() as ctx:
            _emit(ctx, tc, sqrt_c2)
    # bacc passes legalize sync waits (≤1 wait per instruction on TRN2) and
    # insert the activation-table loads.
    nc.compile()
    return nc


def kernel(x, y, proj_w, proj_b, attn_gamma, sum_gamma0, sum_gamma1):
    global LAST_RESULTS
    x = np.asarray(x, dtype=np.float32)
    y = np.asarray(y, dtype=np.float32)
    proj_w = np.asarray(proj_w, dtype=np.float32)
    proj_b = np.asarray(proj_b, dtype=np.float32)
    g0 = math.exp(float(np.asarray(sum_gamma0)))
    g1 = math.exp(float(np.asarray(sum_gamma1)))
    w0 = g0 / (g0 + g1)
    w1 = g1 / (g0 + g1)
    c2 = 1.0 / (SCALE * float(np.asarray(attn_gamma)))

    nc = build_program(math.sqrt(c2))

    def perm(a):
        # [B, H, N, d] -> [B, H, 128, NB, d] partition-major
        return np.ascontiguousarray(
            a.reshape(B, H, NB, 128, a.shape[-1]).transpose(0, 1, 3, 2, 4)
        )

    x_bf = perm(x.astype(ml_dtypes.bfloat16))
    # y with a ones column appended: the O matmul's 65th output row then
    # accumulates the softmax denominators.
    ya = perm(
        np.concatenate(
            [y, np.ones(y.shape[:-1] + (1,), np.float32)], axis=-1
        ).astype(ml_dtypes.bfloat16)
    )
    yb = perm((w0 * y).astype(np.float32))
    # wt rows 0-63 = w1*W^T; row 64 = w1*bias (multiplies the r row, so the
    # 1/r epilogue scale leaves exactly w1*bias). Col 64 = e_r: proj output
    # col 64 of each block passes the denominator row through.
    wt = np.concatenate([proj_w.T * w1, w1 * proj_b[None, :]], axis=0)
    wt = np.concatenate([wt, np.zeros((D + 1, 1), np.float32)], axis=1)
    wt[D, D] = 1.0
    wt = wt.astype(ml_dtypes.bfloat16)

    in_maps = [
        {"x_bf": x_bf[c], "ya": ya[c], "yb": yb[c], "wt": wt}
        for c in range(NCORES)
    ]
    res = run_bass_kernel_spmd(nc, in_maps, list(range(NCORES)))
    LAST_RESULTS = res
    o = np.stack([res.results[c]["out"] for c in range(NCORES)], axis=0)
    # [B, H, 128, NB, D] partition-major -> [B, H, N, D]
    return np.ascontiguousarray(
        o.transpose(0, 1, 3, 2, 4).reshape(B, H, N, D)
    )


# revision 28
# speedup vs baseline: 1.1900x; 1.1900x over previous
"""Trainium2 Bass kernel for nn_Attention_interaction (dense_transformer).

Math (per batch b, head h):
    q = l2norm(x);  S = (q @ q^T) / SCALE / attn_gamma;  P = softmax(S, -1)
    o = P @ y;  o2 = o @ W^T + bias;  out = w0*y + w1*o2
with w_i = exp(sum_gamma_i) / (exp(sum_gamma0) + exp(sum_gamma1)).

Sharding: batch dim B=8 across the 8 cores (1 batch x 8 heads per core).
Per core the 8 heads run in 4 pairs (head A's qT operands on SBUF
partitions 0-63, head B's on 64-127, addressed via matmul tile_position).

The kernel is ACT(exp)-bound: softmax needs 8.4M exps per core; the Scalar
engine runs 1 elem/lane/cycle at 1.2 GHz, so the 64 [128,1024]-chunk
ACTIVATEs floor at ~74us. Everything else is arranged to keep that stream
dense:
  - Softmax skips max-subtraction (q rows are unit vectors so logits are
    bounded) and exp needs no accumulator: softmax denominators accumulate
    in the O matmul's 65th output row via a ones-column appended to y, and
    ride the proj matmul into per-partition layout via a unit column
    appended to the proj weight (out col 64 of each 128-col block = r).
  - Per head the S columns are jc-major and streamed through [128,1024]
    PSUM chunks (2-slot round robin); the two heads' S matmuls interleave
    so their disjoint PE row-quadrants overlap; O matmuls trail one chunk
    so the static per-engine order is always runnable.
  - x for all 8 heads is loaded once; the l2norm scales are computed with
    a handful of wide DVE ops per pair (fast-inverse-sqrt + 2 Newton).
  - All input/output DRAM is partition-major (the host pre/post-permutes)
    so every DMA moves long contiguous runs; y loads and stores ride the
    GpSimd software-DGE queue, the sync HWDGE queue carries the q
    transposes (pair 0's split onto the scalar HWDGE queue too).
  - PE and the exp table are warmed at t=0 with dummy matmuls / a dummy
    ACTIVATE. Pair-0 prep runs at scheduler priority 0, later pairs'
    norms are serialized behind it via bufs=1 scratch tiles, and rs is
    one tile per pair - all to keep the greedy per-engine list scheduler
    from hoisting non-critical work into the fill's dependency chain.
  - proj/denominator/epilogue are split per jc-half and spread across
    chunks (proj0 c=9/10, epilogue0 c=11/12); each pair's jc1 half is
    deferred into the NEXT pair's chunk stream (steps at its c=0..4), so
    pair boundaries cost <1us and only the final pair has a real tail
    (which scales o2 on the by-then-idle Scalar engine, all
    PE/DVE-local, no DRAM hops).
"""

import math
import os

import numpy as np
import ml_dtypes

import concourse.bass as bass
import concourse.bacc as bacc
import concourse.tile as tile
from concourse import mybir
from concourse.bass_utils import run_bass_kernel_spmd
from concourse._compat import get_trn_type

B, H, N, D = 8, 8, 1024, 64
SCALE = (512 // 8) ** (-0.5)  # 0.125
EPS = 1e-6
NCORES = 8
NB = N // 128  # 8 row blocks of 128
NW = N * NB  # 8192 flattened S columns per head
CHUNK = 1024  # exp granularity (PSUM columns per ACT instruction)
F32 = mybir.dt.float32
BF16 = mybir.dt.bfloat16
I32 = mybir.dt.int32
AX = mybir.AxisListType
OP = mybir.AluOpType
ACT = mybir.ActivationFunctionType
MAGIC = 0x5F3759DF

LAST_RESULTS = None  # BassKernelResults of the most recent run (for test.py)


def _emit(ctx, tc, sqrt_c2: float):
    """Emit the per-core program. sqrt_c2 = sqrt(1/(SCALE*attn_gamma)) is
    folded into the q row scales so S comes out of the PE pre-scaled."""
    nc = tc.nc
    # all DRAM I/O is partition-major ([h, p, ...]) so every DMA moves
    # long contiguous runs; the host does the (b p) <-> p b permutes.
    x_bf = nc.dram_tensor("x_bf", [H, 128, NB, D], BF16, kind="ExternalInput")
    ya = nc.dram_tensor("ya", [H, 128, NB, D + 1], BF16, kind="ExternalInput")
    yb = nc.dram_tensor("yb", [H, 128, NB, D], F32, kind="ExternalInput")
    wt = nc.dram_tensor("wt", [D + 1, D + 1], BF16, kind="ExternalInput")
    out = nc.dram_tensor("out", [H, 128, NB, D], F32, kind="ExternalOutput")

    singles = ctx.enter_context(tc.tile_pool(name="singles", bufs=1))
    # bufs=3: ypre(p+1) (emitted at pair p c==0) must not overwrite the
    # buffer pair p-1's tail O-flush is still reading
    io = ctx.enter_context(tc.tile_pool(name="io", bufs=3))
    # bufs=1: successive pairs' norms chains share buffers, so the WAR
    # deps stop the greedy scheduler from hoisting a later pair's norms
    # into the current critical chain
    st = ctx.enter_context(tc.tile_pool(name="st", bufs=1))
    work = ctx.enter_context(tc.tile_pool(name="work", bufs=2))
    epool = ctx.enter_context(tc.tile_pool(name="epool", bufs=2))
    qpool = ctx.enter_context(tc.tile_pool(name="qpool", bufs=1))
    # PSUM: 8 banks = S stream 2x[128,1024] (4) + per-head jc0/jc1 banks (4)
    ps_s = ctx.enter_context(tc.tile_pool(name="ps_s", bufs=2, space="PSUM"))
    ps_o = ctx.enter_context(tc.tile_pool(name="ps_o", bufs=1, space="PSUM"))

    # x for all heads, resident for the whole kernel (8KB/partition);
    # pair 0's heads load first and alone so their transfer isn't queued
    # behind anything
    xall = singles.tile([128, H, NB, D], BF16)
    nc.sync.dma_start(out=xall[:, 0, 0:4], in_=x_bf[0][:, 0:4])
    nc.sync.dma_start(out=xall[:, 1, 0:4], in_=x_bf[1][:, 0:4])
    nc.sync.dma_start(out=xall[:, 0, 4:NB], in_=x_bf[0][:, 4:NB])
    nc.sync.dma_start(out=xall[:, 1, 4:NB], in_=x_bf[1][:, 4:NB])

    # proj weight (rows 0-63 = w1*W^T, row 64 = w1*bias; col 64 = e_r so
    # each proj block's output col 64 is the softmax denominator)
    wt_sb = singles.tile([D + 1, D + 1], BF16)
    nc.sync.dma_start(out=wt_sb, in_=wt[:, :])

    ytiles = {}

    def ypre(p):
        """Prefetch pair p's y operands on the gpsimd software-DGE queue."""
        hA, hB = 2 * p, 2 * p + 1
        yA = io.tile([128, NB, D + 1], BF16, tag="yA", name=f"yA{p}")
        yB = io.tile([128, NB, D + 1], BF16, tag="yB", name=f"yB{p}")
        ybA = io.tile([128, NB, D], F32, tag="ybA", name=f"ybA{p}")
        ybB = io.tile([128, NB, D], F32, tag="ybB", name=f"ybB{p}")
        nc.gpsimd.dma_start(out=yA, in_=ya[hA])
        nc.gpsimd.dma_start(out=yB, in_=ya[hB])
        nc.gpsimd.dma_start(out=ybA, in_=yb[hA])
        nc.gpsimd.dma_start(out=ybB, in_=yb[hB])
        ytiles[p] = (yA, yB, ybA, ybB)

    # row scales rs[p, h, b] = sqrt_c2 / sqrt(sum_d x^2 + eps); one tile
    # per pair so a pair's qscale never waits on later pairs' norms writes
    rs = [
        singles.tile([128, 2, NB], F32, name=f"rs{p}") for p in range(H // 2)
    ]

    # ---- t=0 warmup: exp-table load + PE HAM un-throttle --------------
    scr = singles.tile([128, 2], F32)
    nc.vector.memset(scr, 0.0)
    nc.scalar.activation(out=scr[:, 1:2], in_=scr[:, 0:1], func=ACT.Exp)
    ps_warm = ps_s.tile([128, CHUNK], F32, tag="psS", name="warm")
    for _ in range(60):
        nc.tensor.matmul(
            ps_warm[0:D, 0:D],
            lhsT=wt_sb[0:D, 0:D],
            rhs=wt_sb[0:D, 0:D],
            start=True, stop=True, tile_position=(0, 0),
        )

    def norms(p, b0, nb):
        """l2norm scale for pair p's two heads, blocks b0..b0+nb: a short
        chain of wide DVE ops (square, reduce, fast-inverse-sqrt + 2
        Newton). The last Newton multiply lands directly in rs."""
        hA = 2 * p
        xs = xall[:, hA : hA + 2, b0 : b0 + nb, :]
        rsp = rs[p][:, :, b0 : b0 + nb]
        sq = st.tile([128, 2, NB, D], F32, tag="sq", name="sq")
        sq = sq[:, :, b0 : b0 + nb, :]
        ss = st.tile([128, 2, NB], F32, tag="ss", name="ss")[:, :, b0 : b0 + nb]
        half = st.tile([128, 2, NB], F32, tag="half", name="half")
        half = half[:, :, b0 : b0 + nb]
        yv = st.tile([128, 2, NB], F32, tag="yv", name="yv")[:, :, b0 : b0 + nb]
        t1 = st.tile([128, 2, NB], F32, tag="t1", name="t1")[:, :, b0 : b0 + nb]
        nc.vector.tensor_mul(sq, xs, xs)
        nc.vector.reduce_sum(ss, sq, axis=AX.X)
        nc.vector.tensor_scalar(
            out=half, in0=ss, scalar1=0.5, scalar2=0.5 * EPS,
            op0=OP.mult, op1=OP.add,
        )
        yi = yv.bitcast(I32)
        nc.vector.tensor_scalar(
            out=yi, in0=ss.bitcast(I32), scalar1=1, scalar2=None,
            op0=OP.logical_shift_right,
        )
        nc.vector.tensor_scalar(
            out=yi, in0=yi, scalar1=MAGIC, scalar2=-1,
            op0=OP.subtract, op1=OP.mult,
        )
        for it in range(1):
            last = it == 0
            nc.vector.tensor_mul(t1, yv, yv)
            nc.vector.tensor_mul(t1, t1, half)
            nc.vector.tensor_scalar(
                out=t1, in0=t1, scalar1=1.5,
                scalar2=(-sqrt_c2 if last else -1.0),
                op0=OP.subtract, op1=OP.mult,
            )
            nc.vector.tensor_mul(rsp if last else yv, yv, t1)

    qT = [None] * (H // 2)
    qABs = {}

    def qscale_block(p, b):
        """Scale+cast x block b of pair p into the interleaved [A|B] qAB
        layout (two tensor_scalar ops)."""
        hA, hB = 2 * p, 2 * p + 1
        if p not in qABs:
            qABs[p] = work.tile(
                [128, NB, 128], BF16, tag=f"qAB{p % 2}", name=f"qAB{p}"
            )
        qAB = qABs[p]
        nc.vector.tensor_scalar_mul(
            out=qAB[:, b, 0:D], in0=xall[:, hA, b],
            scalar1=rs[p][:, 0, b : b + 1],
        )
        nc.vector.tensor_scalar_mul(
            out=qAB[:, b, D:128], in0=xall[:, hB, b],
            scalar1=rs[p][:, 1, b : b + 1],
        )

    def qtranspose_block(p, b, engine=None):
        if qT[p] is None:
            qT[p] = qpool.tile([128, N], BF16, tag=f"qT{p}", name=f"qT{p}")
        (engine or nc.sync).dma_start(
            out=qT[p][:, b * 128 : (b + 1) * 128], in_=qABs[p][:, b],
            transpose=True,
        )

    def prep_pair0():
        """Minimal-latency prep of pair 0 at scheduler priority 0, block
        group 0 first (enough for chunk c=0), its transposes split across
        the sync/scalar HWDGE queues."""
        with tc.high_priority():
            norms(0, 0, 4)
            for b in range(4):
                qscale_block(0, b)
                eng = nc.scalar if b % 2 == 1 else nc.sync
                qtranspose_block(0, b, eng)

    prev_tail = None
    prep_pair0()
    ypre(0)
    for h in range(2, H):
        nc.gpsimd.dma_start(out=xall[:, h], in_=x_bf[h])

    for p in range(H // 2):
        hA, hB = 2 * p, 2 * p + 1
        yA, yB, ybA, ybB = ytiles[p]
        q = qT[p]
        # E[p, chunk, head, 512]: chunk c=jc*8+i holds both heads' exp(S)
        # for (i-block rows, jc-half cols)
        E = epool.tile([128, 16, 2, 512], BF16, tag="E", name=f"E{p}")
        OTA = work.tile([D + 1, N], BF16, tag="OTA")
        OTB = work.tile([D + 1, N], BF16, tag="OTB")
        heads = (
            (0, yA, OTA),
            (64, yB, OTB),
        )
        okptr = [0, 0]  # per head: next O matmul (jc-major index jc*8+i)
        otile = [None, None]

        # rtsb[p, jc, head, block-of-half] = softmax denominators
        rtsb = st.tile([128, 2, 2, NB // 2], F32, tag="rtsb")
        rinv = st.tile([128, 2, 2, NB // 2], F32, tag="rinv")

        def emit_o(
            hidx, climit, heads=heads, okptr=okptr, otile=otile, E=E,
            scopy=False,
        ):
            """Emit O matmuls whose E chunk (k < climit) is ready. The
            65th output row accumulates the softmax denominators."""
            base, ytile, OT = heads[hidx]
            hc = "AB"[hidx]
            while okptr[hidx] < min(16, climit):
                k = okptr[hidx]
                jc, i = k // NB, k % NB
                if i == 0:
                    otile[hidx] = ps_o.tile(
                        [128, 512], F32, tag=f"o{jc}{hc}", name=f"ot{jc}{hc}"
                    )
                nc.tensor.matmul(
                    otile[hidx][0 : D + 1, :],
                    lhsT=ytile[:, i, :],
                    rhs=E[:, k, hidx],
                    start=(i == 0), stop=(i == NB - 1), tile_position=(0, 0),
                )
                if i == NB - 1:
                    with tc.high_priority():
                        if scopy:
                            nc.scalar.copy(
                                OT[:, jc * 512 : (jc + 1) * 512],
                                otile[hidx][0 : D + 1, :],
                            )
                        else:
                            nc.vector.tensor_copy(
                                OT[:, jc * 512 : (jc + 1) * 512],
                                otile[hidx][0 : D + 1, :],
                            )
                okptr[hidx] += 1

        def emit_proj(hidx, jc, heads=heads, rtsb=rtsb):
            """proj for output blocks jc*4..jc*4+3 (needs OT cols of that jc
            half); lands in the jc bank this head just freed. Each block's
            output col 64 is the softmax denominator for its 128 tokens."""
            base, ytile, OT = heads[hidx]
            hc = "AB"[hidx]
            pj = ps_o.tile([128, 512], F32, tag=f"o{jc}{hc}", name=f"pj{jc}{hc}")
            for b in range(jc * 4, jc * 4 + 4):
                nc.tensor.matmul(
                    pj[:, (b - jc * 4) * 128 : (b - jc * 4) * 128 + D + 1],
                    lhsT=OT[:, b * 128 : (b + 1) * 128],
                    rhs=wt_sb,
                    start=True, stop=True, tile_position=(0, 0),
                )
            # denominators out of the bank right away (they're per-partition
            # already)
            with tc.high_priority():
                nc.vector.tensor_copy(
                    rtsb[:, jc, hidx],
                    pj.rearrange("p (b c) -> p b c", b=4)[:, :, D],
                )
            return pj

        def emit_rinv(jc, hidx, rtsb=rtsb, rinv=rinv):
            nc.vector.reciprocal(rinv[:, jc, hidx], rtsb[:, jc, hidx])

        pjs = [[None, None], [None, None]]  # [hidx][jc]

        def emit_epilogue(
            hidx, jc, pj, dma=None, hs=None, ybs=None, rinv=rinv, out=out,
            use_act=False,
        ):
            """o2 = pj/r for one jc half, + w0*y, and DMA out. use_act
            scales on the (by then idle) Scalar engine instead of DVE."""
            ho = (hs or (hA, hB))[hidx]
            ybt = (ybs or (ybA, ybB))[hidx]
            o2 = work.tile(
                [128, NB // 2, D], F32, tag=f"o2{'AB'[hidx]}{jc}",
                name=f"o2{'AB'[hidx]}{jc}",
            )
            for b in range(jc * 4, jc * 4 + 4):
                if use_act:
                    nc.scalar.activation(
                        out=o2[:, b - jc * 4, :],
                        in_=pj[:, (b % 4) * 128 : (b % 4) * 128 + D],
                        func=ACT.Copy,
                        scale=rinv[:, jc, hidx, b % 4 : b % 4 + 1],
                    )
                else:
                    nc.vector.tensor_scalar_mul(
                        out=o2[:, b - jc * 4, :],
                        in0=pj[:, (b % 4) * 128 : (b % 4) * 128 + D],
                        scalar1=rinv[:, jc, hidx, b % 4 : b % 4 + 1],
                    )
            fin = work.tile(
                [128, NB // 2, D], F32, tag=f"fin{'AB'[hidx]}{jc}",
                name=f"fin{'AB'[hidx]}{jc}",
            )
            nc.vector.tensor_add(fin, o2, ybt[:, jc * 4 : jc * 4 + 4, :])
            (dma or nc.gpsimd).dma_start(
                out=out[ho][:, jc * 4 : jc * 4 + 4], in_=fin
            )

        # ---- S/exp chunk stream with O interleaved (one-chunk delay) ----
        # Chunk c = jc*8+i holds both heads ([A 512 | B 512]); the two S
        # matmuls hit disjoint PE row-quadrants back-to-back and overlap.
        # Interleaved per chunk: next pair's norms/qscale/transposes and
        # the jc0 denominator/proj/epilogue.
        for c in range(16):
            jc, i = c // NB, c % NB
            ps = ps_s.tile([128, CHUNK], F32, tag="psS", name="psS")
            for hidx in range(2):
                base = heads[hidx][0]
                nc.tensor.matmul(
                    ps[:, hidx * 512 : (hidx + 1) * 512],
                    lhsT=q[base : base + 64, i * 128 : (i + 1) * 128],
                    rhs=q[base : base + 64, jc * 512 : (jc + 1) * 512],
                    start=True, stop=True, tile_position=(base, 0),
                )
            nc.scalar.activation(out=E[:, c], in_=ps, func=ACT.Exp)
            for hidx in range(2):
                emit_o(hidx, c)

            if p == 0:
                # rest of pair 0's own prep (group 1)
                if c == 0:
                    norms(0, 4, 4)
                elif 1 <= c <= 4:
                    qscale_block(0, c + 3)
                    qtranspose_block(0, c + 3)
            if p + 1 < H // 2:
                if c == 0:
                    ypre(p + 1)
                elif c == 5:
                    norms(p + 1, 0, NB)
                elif 6 <= c <= 13:
                    b = c - 6
                    qscale_block(p + 1, b)
                    qtranspose_block(p + 1, b)
            if prev_tail is not None and c <= 4:
                prev_tail(c)
            if c == 9:
                pjs[0][0] = emit_proj(0, 0)
                emit_rinv(0, 0)
            elif c == 10:
                pjs[1][0] = emit_proj(1, 0)
                emit_rinv(0, 1)
            elif c == 11:
                emit_epilogue(0, 0, pjs[0][0])
            elif c == 12:
                emit_epilogue(1, 0, pjs[1][0])

        # ---- jc1 tail: for non-final pairs, deferred into the next
        # pair's chunk stream (steps at its c=0..4) ----
        def make_tail(
            emit_o, emit_proj, emit_rinv, emit_epilogue, pjs,
            hs=(hA, hB), ybs=None,
        ):
            def tail(c):
                if c == 0:
                    emit_o(0, 16)
                    emit_o(1, 16)
                elif c == 1:
                    pjs[0][1] = emit_proj(0, 1)
                    emit_rinv(1, 0)
                elif c == 2:
                    pjs[1][1] = emit_proj(1, 1)
                    emit_rinv(1, 1)
                elif c == 3:
                    emit_epilogue(0, 1, pjs[0][1], hs=hs, ybs=ybs)
                elif c == 4:
                    emit_epilogue(1, 1, pjs[1][1], hs=hs, ybs=ybs)
            return tail

        if p < H // 2 - 1:
            prev_tail = make_tail(
                emit_o, emit_proj, emit_rinv, emit_epilogue, pjs,
                hs=(hA, hB), ybs=(ybA, ybB),
            )
        else:
            emit_o(0, 16)
            emit_o(1, 16, scopy=True)
            pjs[0][1] = emit_proj(0, 1)
            emit_rinv(1, 0)
            pjs[1][1] = emit_proj(1, 1)
            emit_rinv(1, 1)
            emit_epilogue(0, 1, pjs[0][1], dma=nc.sync, use_act=True)
            emit_epilogue(1, 1, pjs[1][1], dma=nc.sync)


def build_program(sqrt_c2: float) -> bass.Bass:
    from contextlib import ExitStack

    nc = bacc.Bacc(get_trn_type() or "TRN2", target_bir_lowering=False)
    with tile.TileContext(nc) as tc:
        with ExitStack() as ctx:
            _emit(ctx, tc, sqrt_c2)
    # bacc passes legalize sync waits (≤1 wait per instruction on TRN2) and
    # insert the activation-table loads.
    nc.compile()
    return nc


def kernel(x, y, proj_w, proj_b, attn_gamma, sum_gamma0, sum_gamma1):
    global LAST_RESULTS
    x = np.asarray(x, dtype=np.float32)
    y = np.asarray(y, dtype=np.float32)
    proj_w = np.asarray(proj_w, dtype=np.float32)
    proj_b = np.asarray(proj_b, dtype=np.float32)
    g0 = math.exp(float(np.asarray(sum_gamma0)))
    g1 = math.exp(float(np.asarray(sum_gamma1)))
    w0 = g0 / (g0 + g1)
    w1 = g1 / (g0 + g1)
    c2 = 1.0 / (SCALE * float(np.asarray(attn_gamma)))

    nc = build_program(math.sqrt(c2))

    def perm(a):
        # [B, H, N, d] -> [B, H, 128, NB, d] partition-major
        return np.ascontiguousarray(
            a.reshape(B, H, NB, 128, a.shape[-1]).transpose(0, 1, 3, 2, 4)
        )

    x_bf = perm(x.astype(ml_dtypes.bfloat16))
    # y with a ones column appended: the O matmul's 65th output row then
    # accumulates the softmax denominators.
    ya = perm(
        np.concatenate(
            [y, np.ones(y.shape[:-1] + (1,), np.float32)], axis=-1
        ).astype(ml_dtypes.bfloat16)
    )
    yb = perm((w0 * y).astype(np.float32))
    # wt rows 0-63 = w1*W^T; row 64 = w1*bias (multiplies the r row, so the
    # 1/r epilogue scale leaves exactly w1*bias). Col 64 = e_r: proj output
    # col 64 of each block passes the denominator row through.
    wt = np.concatenate([proj_w.T * w1, w1 * proj_b[None, :]], axis=0)
    wt = np.concatenate([wt, np.zeros((D + 1, 1), np.float32)], axis=1)
    wt[D, D] = 1.0
    wt = wt.astype(ml_dtypes.bfloat16)

    in_maps = [
        {"x_bf": x_bf[c], "ya": ya[c], "yb": yb[c], "wt": wt}
        for c in range(NCORES)
    ]
    res = run_bass_kernel_spmd(nc, in_maps, list(range(NCORES)))
    LAST_RESULTS = res
    o = np.stack([res.results[c]["out"] for c in range(NCORES)], axis=0)
    # [B, H, 128, NB, D] partition-major -> [B, H, N, D]
    return np.ascontiguousarray(
        o.transpose(0, 1, 3, 2, 4).reshape(B, H, N, D)
    )


# revision 29
# speedup vs baseline: 1.1971x; 1.0060x over previous
"""Trainium2 Bass kernel for nn_Attention_interaction (dense_transformer).

Math (per batch b, head h):
    q = l2norm(x);  S = (q @ q^T) / SCALE / attn_gamma;  P = softmax(S, -1)
    o = P @ y;  o2 = o @ W^T + bias;  out = w0*y + w1*o2
with w_i = exp(sum_gamma_i) / (exp(sum_gamma0) + exp(sum_gamma1)).

Sharding: batch dim B=8 across the 8 cores (1 batch x 8 heads per core).
Per core the 8 heads run in 4 pairs (head A's qT operands on SBUF
partitions 0-63, head B's on 64-127, addressed via matmul tile_position).

The kernel is ACT(exp)-bound: softmax needs 8.4M exps per core; the Scalar
engine runs 1 elem/lane/cycle at 1.2 GHz, so the 64 [128,1024]-chunk
ACTIVATEs floor at ~74us. Everything else is arranged to keep that stream
dense:
  - Softmax skips max-subtraction (q rows are unit vectors so logits are
    bounded) and exp needs no accumulator: softmax denominators accumulate
    in the O matmul's 65th output row via a ones-column appended to y, and
    ride the proj matmul into per-partition layout via a unit column
    appended to the proj weight (out col 64 of each 128-col block = r).
  - Per head the S columns are jc-major and streamed through [128,1024]
    PSUM chunks (2-slot round robin); the two heads' S matmuls interleave
    so their disjoint PE row-quadrants overlap; O matmuls trail one chunk
    so the static per-engine order is always runnable.
  - x for all 8 heads is loaded once; the l2norm scales are computed with
    a handful of wide DVE ops per pair (fast-inverse-sqrt + 2 Newton).
  - All input/output DRAM is partition-major (the host pre/post-permutes)
    so every DMA moves long contiguous runs; y loads and stores ride the
    GpSimd software-DGE queue, the sync HWDGE queue carries the q
    transposes (pair 0's split onto the scalar HWDGE queue too).
  - PE and the exp table are warmed at t=0 with dummy matmuls / a dummy
    ACTIVATE. Pair-0 prep runs at scheduler priority 0, later pairs'
    norms are serialized behind it via bufs=1 scratch tiles, and rs is
    one tile per pair - all to keep the greedy per-engine list scheduler
    from hoisting non-critical work into the fill's dependency chain.
  - proj/denominator/epilogue are split per jc-half and spread across
    chunks (proj0 c=9/10, epilogue0 c=11/12); each pair's jc1 half is
    deferred into the NEXT pair's chunk stream (steps at its c=0..4), so
    pair boundaries cost <1us and only the final pair has a real tail
    (which scales o2 on the by-then-idle Scalar engine, all
    PE/DVE-local, no DRAM hops).
"""

import math
import os

import numpy as np
import ml_dtypes

import concourse.bass as bass
import concourse.bacc as bacc
import concourse.tile as tile
from concourse import mybir
from concourse.bass_utils import run_bass_kernel_spmd
from concourse._compat import get_trn_type

B, H, N, D = 8, 8, 1024, 64
SCALE = (512 // 8) ** (-0.5)  # 0.125
EPS = 1e-6
NCORES = 8
NB = N // 128  # 8 row blocks of 128
NW = N * NB  # 8192 flattened S columns per head
CHUNK = 1024  # exp granularity (PSUM columns per ACT instruction)
F32 = mybir.dt.float32
BF16 = mybir.dt.bfloat16
I32 = mybir.dt.int32
AX = mybir.AxisListType
OP = mybir.AluOpType
ACT = mybir.ActivationFunctionType
MAGIC = 0x5F3759DF

LAST_RESULTS = None  # BassKernelResults of the most recent run (for test.py)


def _emit(ctx, tc, sqrt_c2: float):
    """Emit the per-core program. sqrt_c2 = sqrt(1/(SCALE*attn_gamma)) is
    folded into the q row scales so S comes out of the PE pre-scaled."""
    nc = tc.nc
    # all DRAM I/O is partition-major ([h, p, ...]) so every DMA moves
    # long contiguous runs; the host does the (b p) <-> p b permutes.
    x_bf = nc.dram_tensor("x_bf", [H, 128, NB, D], BF16, kind="ExternalInput")
    ya = nc.dram_tensor("ya", [H, 128, NB, D + 1], BF16, kind="ExternalInput")
    yb = nc.dram_tensor("yb", [H, 128, NB, D], F32, kind="ExternalInput")
    wt = nc.dram_tensor("wt", [D + 1, D + 1], BF16, kind="ExternalInput")
    out = nc.dram_tensor("out", [H, 128, NB, D], F32, kind="ExternalOutput")

    singles = ctx.enter_context(tc.tile_pool(name="singles", bufs=1))
    # bufs=3: ypre(p+1) (emitted at pair p c==0) must not overwrite the
    # buffer pair p-1's tail O-flush is still reading
    io = ctx.enter_context(tc.tile_pool(name="io", bufs=3))
    # bufs=1: successive pairs' norms chains share buffers, so the WAR
    # deps stop the greedy scheduler from hoisting a later pair's norms
    # into the current critical chain
    st = ctx.enter_context(tc.tile_pool(name="st", bufs=1))
    work = ctx.enter_context(tc.tile_pool(name="work", bufs=2))
    epool = ctx.enter_context(tc.tile_pool(name="epool", bufs=2))
    qpool = ctx.enter_context(tc.tile_pool(name="qpool", bufs=1))
    # PSUM: 8 banks = S stream 2x[128,1024] (4) + per-head jc0/jc1 banks (4)
    ps_s = ctx.enter_context(tc.tile_pool(name="ps_s", bufs=2, space="PSUM"))
    ps_o = ctx.enter_context(tc.tile_pool(name="ps_o", bufs=1, space="PSUM"))

    # x for all heads, resident for the whole kernel (8KB/partition);
    # pair 0's heads load first and alone so their transfer isn't queued
    # behind anything
    xall = singles.tile([128, H, NB, D], BF16)
    nc.sync.dma_start(out=xall[:, 0, 0:4], in_=x_bf[0][:, 0:4])
    nc.sync.dma_start(out=xall[:, 1, 0:4], in_=x_bf[1][:, 0:4])
    nc.sync.dma_start(out=xall[:, 0, 4:NB], in_=x_bf[0][:, 4:NB])
    nc.sync.dma_start(out=xall[:, 1, 4:NB], in_=x_bf[1][:, 4:NB])

    # proj weight (rows 0-63 = w1*W^T, row 64 = w1*bias; col 64 = e_r so
    # each proj block's output col 64 is the softmax denominator)
    wt_sb = singles.tile([D + 1, D + 1], BF16)
    nc.sync.dma_start(out=wt_sb, in_=wt[:, :])

    ytiles = {}

    def ypre(p):
        """Prefetch pair p's y operands on the gpsimd software-DGE queue."""
        hA, hB = 2 * p, 2 * p + 1
        yA = io.tile([128, NB, D + 1], BF16, tag="yA", name=f"yA{p}")
        yB = io.tile([128, NB, D + 1], BF16, tag="yB", name=f"yB{p}")
        ybA = io.tile([128, NB, D], F32, tag="ybA", name=f"ybA{p}")
        ybB = io.tile([128, NB, D], F32, tag="ybB", name=f"ybB{p}")
        nc.gpsimd.dma_start(out=yA, in_=ya[hA])
        nc.gpsimd.dma_start(out=yB, in_=ya[hB])
        nc.gpsimd.dma_start(out=ybA, in_=yb[hA])
        nc.gpsimd.dma_start(out=ybB, in_=yb[hB])
        ytiles[p] = (yA, yB, ybA, ybB)

    # row scales rs[p, h, b] = sqrt_c2 / sqrt(sum_d x^2 + eps); one tile
    # per pair so a pair's qscale never waits on later pairs' norms writes
    rs = [
        singles.tile([128, 2, NB], F32, name=f"rs{p}") for p in range(H // 2)
    ]

    # ---- t=0 warmup: exp-table load + PE HAM un-throttle --------------
    scr = singles.tile([128, 2], F32)
    nc.vector.memset(scr, 0.0)
    nc.scalar.activation(out=scr[:, 1:2], in_=scr[:, 0:1], func=ACT.Exp)

    def norms(p, b0, nb):
        """l2norm scale for pair p's two heads, blocks b0..b0+nb: a short
        chain of wide DVE ops (square, reduce, fast-inverse-sqrt + 2
        Newton). The last Newton multiply lands directly in rs."""
        hA = 2 * p
        xs = xall[:, hA : hA + 2, b0 : b0 + nb, :]
        rsp = rs[p][:, :, b0 : b0 + nb]
        sq = st.tile([128, 2, NB, D], F32, tag="sq", name="sq")
        sq = sq[:, :, b0 : b0 + nb, :]
        ss = st.tile([128, 2, NB], F32, tag="ss", name="ss")[:, :, b0 : b0 + nb]
        half = st.tile([128, 2, NB], F32, tag="half", name="half")
        half = half[:, :, b0 : b0 + nb]
        yv = st.tile([128, 2, NB], F32, tag="yv", name="yv")[:, :, b0 : b0 + nb]
        t1 = st.tile([128, 2, NB], F32, tag="t1", name="t1")[:, :, b0 : b0 + nb]
        nc.vector.tensor_mul(sq, xs, xs)
        nc.vector.reduce_sum(ss, sq, axis=AX.X)
        nc.vector.tensor_scalar(
            out=half, in0=ss, scalar1=0.5, scalar2=0.5 * EPS,
            op0=OP.mult, op1=OP.add,
        )
        yi = yv.bitcast(I32)
        nc.vector.tensor_scalar(
            out=yi, in0=ss.bitcast(I32), scalar1=1, scalar2=None,
            op0=OP.logical_shift_right,
        )
        nc.vector.tensor_scalar(
            out=yi, in0=yi, scalar1=MAGIC, scalar2=-1,
            op0=OP.subtract, op1=OP.mult,
        )
        for it in range(1):
            last = it == 0
            nc.vector.tensor_mul(t1, yv, yv)
            nc.vector.tensor_mul(t1, t1, half)
            nc.vector.tensor_scalar(
                out=t1, in0=t1, scalar1=1.5,
                scalar2=(-sqrt_c2 if last else -1.0),
                op0=OP.subtract, op1=OP.mult,
            )
            nc.vector.tensor_mul(rsp if last else yv, yv, t1)

    qT = [None] * (H // 2)
    qABs = {}

    def qscale_block(p, b):
        """Scale+cast x block b of pair p into the interleaved [A|B] qAB
        layout (two tensor_scalar ops)."""
        hA, hB = 2 * p, 2 * p + 1
        if p not in qABs:
            qABs[p] = work.tile(
                [128, NB, 128], BF16, tag=f"qAB{p % 2}", name=f"qAB{p}"
            )
        qAB = qABs[p]
        nc.vector.tensor_scalar_mul(
            out=qAB[:, b, 0:D], in0=xall[:, hA, b],
            scalar1=rs[p][:, 0, b : b + 1],
        )
        nc.vector.tensor_scalar_mul(
            out=qAB[:, b, D:128], in0=xall[:, hB, b],
            scalar1=rs[p][:, 1, b : b + 1],
        )

    def qtranspose_block(p, b, engine=None):
        if qT[p] is None:
            qT[p] = qpool.tile([128, N], BF16, tag=f"qT{p}", name=f"qT{p}")
        (engine or nc.sync).dma_start(
            out=qT[p][:, b * 128 : (b + 1) * 128], in_=qABs[p][:, b],
            transpose=True,
        )

    def prep_pair0():
        """Minimal-latency prep of pair 0 at scheduler priority 0, block
        group 0 first (enough for chunk c=0), its transposes split across
        the sync/scalar HWDGE queues."""
        with tc.high_priority():
            norms(0, 0, 4)
            for b in range(4):
                qscale_block(0, b)
                eng = nc.scalar if b % 2 == 1 else nc.sync
                qtranspose_block(0, b, eng)

    prev_tail = None
    prep_pair0()
    ypre(0)
    for h in range(2, H):
        nc.gpsimd.dma_start(out=xall[:, h], in_=x_bf[h])

    for p in range(H // 2):
        hA, hB = 2 * p, 2 * p + 1
        yA, yB, ybA, ybB = ytiles[p]
        q = qT[p]
        # E[p, chunk, head, 512]: chunk c=jc*8+i holds both heads' exp(S)
        # for (i-block rows, jc-half cols)
        E = epool.tile([128, 16, 2, 512], BF16, tag="E", name=f"E{p}")
        OTA = work.tile([D + 1, N], BF16, tag="OTA")
        OTB = work.tile([D + 1, N], BF16, tag="OTB")
        heads = (
            (0, yA, OTA),
            (64, yB, OTB),
        )
        okptr = [0, 0]  # per head: next O matmul (jc-major index jc*8+i)
        otile = [None, None]

        # rtsb[p, jc, head, block-of-half] = softmax denominators
        rtsb = st.tile([128, 2, 2, NB // 2], F32, tag="rtsb")
        rinv = st.tile([128, 2, 2, NB // 2], F32, tag="rinv")

        def emit_o(
            hidx, climit, heads=heads, okptr=okptr, otile=otile, E=E,
            scopy=False,
        ):
            """Emit O matmuls whose E chunk (k < climit) is ready. The
            65th output row accumulates the softmax denominators."""
            base, ytile, OT = heads[hidx]
            hc = "AB"[hidx]
            while okptr[hidx] < min(16, climit):
                k = okptr[hidx]
                jc, i = k // NB, k % NB
                if i == 0:
                    otile[hidx] = ps_o.tile(
                        [128, 512], F32, tag=f"o{jc}{hc}", name=f"ot{jc}{hc}"
                    )
                nc.tensor.matmul(
                    otile[hidx][0 : D + 1, :],
                    lhsT=ytile[:, i, :],
                    rhs=E[:, k, hidx],
                    start=(i == 0), stop=(i == NB - 1), tile_position=(0, 0),
                )
                if i == NB - 1:
                    with tc.high_priority():
                        if scopy:
                            nc.scalar.copy(
                                OT[:, jc * 512 : (jc + 1) * 512],
                                otile[hidx][0 : D + 1, :],
                            )
                        else:
                            nc.vector.tensor_copy(
                                OT[:, jc * 512 : (jc + 1) * 512],
                                otile[hidx][0 : D + 1, :],
                            )
                okptr[hidx] += 1

        def emit_proj(hidx, jc, heads=heads, rtsb=rtsb):
            """proj for output blocks jc*4..jc*4+3 (needs OT cols of that jc
            half); lands in the jc bank this head just freed. Each block's
            output col 64 is the softmax denominator for its 128 tokens."""
            base, ytile, OT = heads[hidx]
            hc = "AB"[hidx]
            pj = ps_o.tile([128, 512], F32, tag=f"o{jc}{hc}", name=f"pj{jc}{hc}")
            for b in range(jc * 4, jc * 4 + 4):
                nc.tensor.matmul(
                    pj[:, (b - jc * 4) * 128 : (b - jc * 4) * 128 + D + 1],
                    lhsT=OT[:, b * 128 : (b + 1) * 128],
                    rhs=wt_sb,
                    start=True, stop=True, tile_position=(0, 0),
                )
            # denominators out of the bank right away (they're per-partition
            # already)
            with tc.high_priority():
                nc.vector.tensor_copy(
                    rtsb[:, jc, hidx],
                    pj.rearrange("p (b c) -> p b c", b=4)[:, :, D],
                )
            return pj

        def emit_rinv(jc, hidx, rtsb=rtsb, rinv=rinv):
            nc.vector.reciprocal(rinv[:, jc, hidx], rtsb[:, jc, hidx])

        pjs = [[None, None], [None, None]]  # [hidx][jc]

        def emit_epilogue(
            hidx, jc, pj, dma=None, hs=None, ybs=None, rinv=rinv, out=out,
            use_act=False,
        ):
            """o2 = pj/r for one jc half, + w0*y, and DMA out. use_act
            scales on the (by then idle) Scalar engine instead of DVE."""
            ho = (hs or (hA, hB))[hidx]
            ybt = (ybs or (ybA, ybB))[hidx]
            o2 = work.tile(
                [128, NB // 2, D], F32, tag=f"o2{'AB'[hidx]}{jc}",
                name=f"o2{'AB'[hidx]}{jc}",
            )
            for b in range(jc * 4, jc * 4 + 4):
                if use_act:
                    nc.scalar.activation(
                        out=o2[:, b - jc * 4, :],
                        in_=pj[:, (b % 4) * 128 : (b % 4) * 128 + D],
                        func=ACT.Copy,
                        scale=rinv[:, jc, hidx, b % 4 : b % 4 + 1],
                    )
                else:
                    nc.vector.tensor_scalar_mul(
                        out=o2[:, b - jc * 4, :],
                        in0=pj[:, (b % 4) * 128 : (b % 4) * 128 + D],
                        scalar1=rinv[:, jc, hidx, b % 4 : b % 4 + 1],
                    )
            fin = work.tile(
                [128, NB // 2, D], F32, tag=f"fin{'AB'[hidx]}{jc}",
                name=f"fin{'AB'[hidx]}{jc}",
            )
            nc.vector.tensor_add(fin, o2, ybt[:, jc * 4 : jc * 4 + 4, :])
            (dma or nc.gpsimd).dma_start(
                out=out[ho][:, jc * 4 : jc * 4 + 4], in_=fin
            )

        # ---- S/exp chunk stream with O interleaved (one-chunk delay) ----
        # Chunk c = jc*8+i holds both heads ([A 512 | B 512]); the two S
        # matmuls hit disjoint PE row-quadrants back-to-back and overlap.
        # Interleaved per chunk: next pair's norms/qscale/transposes and
        # the jc0 denominator/proj/epilogue.
        for c in range(16):
            jc, i = c // NB, c % NB
            ps = ps_s.tile([128, CHUNK], F32, tag="psS", name="psS")
            for hidx in range(2):
                base = heads[hidx][0]
                nc.tensor.matmul(
                    ps[:, hidx * 512 : (hidx + 1) * 512],
                    lhsT=q[base : base + 64, i * 128 : (i + 1) * 128],
                    rhs=q[base : base + 64, jc * 512 : (jc + 1) * 512],
                    start=True, stop=True, tile_position=(base, 0),
                )
            nc.scalar.activation(out=E[:, c], in_=ps, func=ACT.Exp)
            for hidx in range(2):
                emit_o(hidx, c)

            if p == 0:
                # rest of pair 0's own prep (group 1)
                if c == 0:
                    norms(0, 4, 4)
                elif 1 <= c <= 4:
                    qscale_block(0, c + 3)
                    qtranspose_block(0, c + 3)
            if p + 1 < H // 2:
                if c == 0:
                    ypre(p + 1)
                elif c == 5:
                    norms(p + 1, 0, NB)
                elif 6 <= c <= 13:
                    b = c - 6
                    qscale_block(p + 1, b)
                    qtranspose_block(p + 1, b)
            if prev_tail is not None and c <= 4:
                prev_tail(c)
            if c == 9:
                pjs[0][0] = emit_proj(0, 0)
                emit_rinv(0, 0)
            elif c == 10:
                pjs[1][0] = emit_proj(1, 0)
                emit_rinv(0, 1)
            elif c == 11:
                emit_epilogue(0, 0, pjs[0][0])
            elif c == 12:
                emit_epilogue(1, 0, pjs[1][0])

        # ---- jc1 tail: for non-final pairs, deferred into the next
        # pair's chunk stream (steps at its c=0..4) ----
        def make_tail(
            emit_o, emit_proj, emit_rinv, emit_epilogue, pjs,
            hs=(hA, hB), ybs=None,
        ):
            def tail(c):
                if c == 0:
                    emit_o(0, 16)
                    emit_o(1, 16)
                elif c == 1:
                    pjs[0][1] = emit_proj(0, 1)
                    emit_rinv(1, 0)
                elif c == 2:
                    pjs[1][1] = emit_proj(1, 1)
                    emit_rinv(1, 1)
                elif c == 3:
                    emit_epilogue(0, 1, pjs[0][1], hs=hs, ybs=ybs)
                elif c == 4:
                    emit_epilogue(1, 1, pjs[1][1], hs=hs, ybs=ybs)
            return tail

        if p < H // 2 - 1:
            prev_tail = make_tail(
                emit_o, emit_proj, emit_rinv, emit_epilogue, pjs,
                hs=(hA, hB), ybs=(ybA, ybB),
            )
        else:
            emit_o(0, 16)
            emit_o(1, 16, scopy=True)
            pjs[0][1] = emit_proj(0, 1)
            emit_rinv(1, 0)
            pjs[1][1] = emit_proj(1, 1)
            emit_rinv(1, 1)
            emit_epilogue(0, 1, pjs[0][1], dma=nc.sync, use_act=True)
            emit_epilogue(1, 1, pjs[1][1], dma=nc.sync)


def build_program(sqrt_c2: float) -> bass.Bass:
    from contextlib import ExitStack

    nc = bacc.Bacc(get_trn_type() or "TRN2", target_bir_lowering=False)
    with tile.TileContext(nc) as tc:
        with ExitStack() as ctx:
            _emit(ctx, tc, sqrt_c2)
    # bacc passes legalize sync waits (≤1 wait per instruction on TRN2) and
    # insert the activation-table loads.
    nc.compile()
    return nc


def kernel(x, y, proj_w, proj_b, attn_gamma, sum_gamma0, sum_gamma1):
    global LAST_RESULTS
    x = np.asarray(x, dtype=np.float32)
    y = np.asarray(y, dtype=np.float32)
    proj_w = np.asarray(proj_w, dtype=np.float32)
    proj_b = np.asarray(proj_b, dtype=np.float32)
    g0 = math.exp(float(np.asarray(sum_gamma0)))
    g1 = math.exp(float(np.asarray(sum_gamma1)))
    w0 = g0 / (g0 + g1)
    w1 = g1 / (g0 + g1)
    c2 = 1.0 / (SCALE * float(np.asarray(attn_gamma)))

    nc = build_program(math.sqrt(c2))

    def perm(a):
        # [B, H, N, d] -> [B, H, 128, NB, d] partition-major
        return np.ascontiguousarray(
            a.reshape(B, H, NB, 128, a.shape[-1]).transpose(0, 1, 3, 2, 4)
        )

    x_bf = perm(x.astype(ml_dtypes.bfloat16))
    # y with a ones column appended: the O matmul's 65th output row then
    # accumulates the softmax denominators.
    ya = perm(
        np.concatenate(
            [y, np.ones(y.shape[:-1] + (1,), np.float32)], axis=-1
        ).astype(ml_dtypes.bfloat16)
    )
    yb = perm((w0 * y).astype(np.float32))
    # wt rows 0-63 = w1*W^T; row 64 = w1*bias (multiplies the r row, so the
    # 1/r epilogue scale leaves exactly w1*bias). Col 64 = e_r: proj output
    # col 64 of each block passes the denominator row through.
    wt = np.concatenate([proj_w.T * w1, w1 * proj_b[None, :]], axis=0)
    wt = np.concatenate([wt, np.zeros((D + 1, 1), np.float32)], axis=1)
    wt[D, D] = 1.0
    wt = wt.astype(ml_dtypes.bfloat16)

    in_maps = [
        {"x_bf": x_bf[c], "ya": ya[c], "yb": yb[c], "wt": wt}
        for c in range(NCORES)
    ]
    res = run_bass_kernel_spmd(nc, in_maps, list(range(NCORES)))
    LAST_RESULTS = res
    o = np.stack([res.results[c]["out"] for c in range(NCORES)], axis=0)
    # [B, H, 128, NB, D] partition-major -> [B, H, N, D]
    return np.ascontiguousarray(
        o.transpose(0, 1, 3, 2, 4).reshape(B, H, N, D)
    )
